# revision 2
# baseline (speedup 1.0000x reference)
"""VQ codebook kernel v4 for Trainium2, 8 NeuronCores.

Problem: x (64, 64, 32, 32) f32, codebook (512, 64) f32.
  idx = argmin_k ||flat - codebook[k]||^2 (first-min), out = codebook[idx].

Device computes argmin indices; host gathers codebook rows (0.01% of the
FLOPs, memory-bound on host, exact).

Sharding: data-parallel over batch (8 batches/core), codebook replicated.

Per-core pipeline, 64 token-tiles of 128 tokens, in quads (2x4 PSUM banks):
  PE:  score s = x.e - |e|^2/2 in bf16 hi/lo (3-term, proven <=1 flip):
       MM1 [xh;xl]x[eh;eh] + MM2a xh x el + MM2b ones2 x [e2a;e2b]
       (MM2a/b concurrent via tile_position row groups).
  DVE: r = prefix-max scan of s along k (tensor_tensor_scan, PSUM src,
       the ONLY DVE touch - no reduce, no match).
  ACT: idx = accum Sign(-r + r[:,511]) = #(k : r_k < max) = FIRST argmax,
       via the scalar engine's sum-accumulator. Exact (f32 compares,
       +-1/0 sums).
  One final DMA of idx [128, 64] f32 per core.
"""

import numpy as np
import ml_dtypes

import concourse.bass as bass
import concourse.mybir as mybir
import concourse.tile as tile
from concourse.bass_utils import run_bass_kernel_spmd

N_CORES = 8
B_FULL, D, H, W = 64, 64, 32, 32
N = H * W          # 1024 tokens per batch
K = 512            # codebook entries
B_CORE = B_FULL // N_CORES   # 8 batches per core
TOK_TILE = 128
TILES_PER_BATCH = N // TOK_TILE          # 8
N_TILES = B_CORE * TILES_PER_BATCH       # 64 per core
QUAD = 4

F32 = mybir.dt.float32
FP16 = mybir.dt.float16
BF16 = mybir.dt.bfloat16

MAX_SYNC_WAITS = 1


def _split_excess_waits(nc, max_waits=MAX_SYNC_WAITS):
    """This container's walrus rejects >2 sync waits per instruction; move
    excess waits onto InstNoOp instructions inserted just before."""
    for f in nc.m.functions:
        for bb in f.blocks:
            new_list = []
            for inst in bb.instructions:
                si = inst.sync_info
                if si is not None and si.on_wait and len(si.on_wait) > max_waits:
                    waits = list(si.on_wait)
                    extra, keep = waits[:-max_waits], waits[-max_waits:]
                    for i, w in enumerate(extra):
                        nop = mybir.InstNoOp(name=f"{inst.name}-sw{i}", ins=[], outs=[])
                        nop.engine = inst.engine
                        nop.sync_info = mybir.SyncInfo(on_wait=[w], on_update=[])
                        new_list.append(nop)
                    si.on_wait = keep
                    new_list.append(inst)
                else:
                    new_list.append(inst)
            bb.instructions[:] = new_list


def _emit_body(nc, pools, x_hl, r1, r2, ones2, ones32, cntbuf, rep):
    sb, dlp, ps = pools
    for q in range(N_TILES // QUAD):
        tiles = [QUAD * q + h for h in range(QUAD)]
        ps4 = ps.tile([TOK_TILE, QUAD, K], F32, tag="ps4")
        b, j0 = divmod(tiles[0], TILES_PER_BATCH)
        xt4 = sb.tile([128, QUAD * TOK_TILE], BF16, tag="xt4")
        nc.sync.dma_start(
            out=xt4[:], in_=x_hl[b, :, j0 * TOK_TILE:(j0 + QUAD) * TOK_TILE])
        for h, t in enumerate(tiles):
            xt = xt4[:, h * TOK_TILE:(h + 1) * TOK_TILE]
            nc.tensor.matmul(ps4[:, h, :], lhsT=xt, rhs=r1[:],
                             start=True, stop=False)
            nc.tensor.matmul(ps4[:, h, :], lhsT=xt[0:D, :], rhs=r2[0:D, :],
                             start=False, stop=False, tile_position=(0, 0))
            nc.tensor.matmul(ps4[:, h, :], lhsT=ones2[96:98, :],
                             rhs=r2[96:98, :],
                             start=False, stop=True, tile_position=(96, 0))

        for h, t in enumerate(tiles):
            r = dlp.tile([TOK_TILE, K], F32, tag="r")
            nc.vector.tensor_tensor_scan(
                out=r[:], data0=ps4[:, h, :], data1=ones32[:],
                initial=-3.0e38,
                op0=mybir.AluOpType.max, op1=mybir.AluOpType.mult)
            sg = dlp.tile([TOK_TILE, K], FP16, tag="sg")
            nc.scalar.activation(sg[:], r[:],
                                 mybir.ActivationFunctionType.Sign,
                                 bias=r[:, K - 1:K], scale=-1.0,
                                 accum_out=cntbuf[:, t:t + 1])


def build_nc(reps=1):
    nc = bass.Bass()
    x_hl = nc.dram_tensor("x_hl", [B_CORE, 128, N], BF16, kind="ExternalInput")
    r1d = nc.dram_tensor("r1d", [128, K], BF16, kind="ExternalInput")
    r2d = nc.dram_tensor("r2d", [98, K], BF16, kind="ExternalInput")
    out = nc.dram_tensor("out", [TOK_TILE, N_TILES], F32,
                         kind="ExternalOutput")

    with tile.TileContext(nc) as tc:
        with (
            tc.tile_pool(name="const", bufs=1) as constp,
            tc.tile_pool(name="sbuf", bufs=8) as sb,
            tc.tile_pool(name="delta", bufs=8) as dlp,
            tc.tile_pool(name="psum", bufs=2, space="PSUM") as ps,
        ):
            r1 = constp.tile([128, K], BF16)
            nc.sync.dma_start(out=r1[:], in_=r1d[:])
            r2 = constp.tile([98, K], BF16)
            nc.sync.dma_start(out=r2[:], in_=r2d[:])
            ones2 = constp.tile([98, TOK_TILE], BF16)
            nc.gpsimd.memset(ones2[:], 1.0)
            ones32 = constp.tile([TOK_TILE, K], F32)
            nc.gpsimd.memset(ones32[:], 1.0)

            pools = (sb, dlp, ps)
            for rep in range(reps):
                cntbuf = constp.tile([TOK_TILE, N_TILES], F32,
                                     tag=f"cntbuf{rep}")
                _emit_body(nc, pools, x_hl, r1, r2, ones2, ones32, cntbuf,
                           rep)
                nc.sync.dma_start(out=out[:], in_=cntbuf[:])

    _split_excess_waits(nc)
    return nc


_NC_CACHE = None


def _get_nc():
    global _NC_CACHE
    if _NC_CACHE is None:
        _NC_CACHE = build_nc()
    return _NC_CACHE


def prep_inputs(x, codebook):
    """Host-side prep: shard x over batch; bf16 hi/lo split of x; build
    rhs constants [eh;eh], [el ; pad ; e2a;e2b]."""
    bf = ml_dtypes.bfloat16
    x = np.asarray(x, dtype=np.float32).reshape(B_FULL, D, N)
    cbk = np.asarray(codebook, dtype=np.float32)

    eh = cbk.astype(bf).astype(np.float32)
    el = (cbk - eh).astype(bf).astype(np.float32)
    e2 = (cbk.astype(np.float64) ** 2).sum(-1)
    e2h32 = (-0.5 * e2).astype(np.float32)
    e2a = e2h32.astype(bf).astype(np.float32)
    e2b = (e2h32 - e2a).astype(np.float32)

    r1 = np.concatenate([eh.T, eh.T], axis=0).astype(bf)          # [128, K]
    r2 = np.zeros((98, K), dtype=np.float32)
    r2[0:D] = el.T
    r2[96] = e2a
    r2[97] = e2b
    r2 = r2.astype(bf)

    in_maps = []
    for c in range(N_CORES):
        xs = x[c * B_CORE:(c + 1) * B_CORE]                       # [8, 64, 1024]
        xh = xs.astype(bf).astype(np.float32)
        xl = (xs - xh)
        x_hl = np.concatenate([xh, xl], axis=1).astype(bf)        # [8, 128, 1024]
        in_maps.append({"x_hl": np.ascontiguousarray(x_hl), "r1d": r1,
                        "r2d": r2})
    return in_maps


def kernel(x, codebook):
    nc = _get_nc()
    cbk = np.asarray(codebook, dtype=np.float32)
    in_maps = prep_inputs(x, codebook)
    res = run_bass_kernel_spmd(nc, in_maps, core_ids=list(range(N_CORES)))
    # out[c] is [128, 64] f32: count (= argmin idx) of token p, tile t
    idx = np.stack([r["out"] for r in res.results], axis=0)   # [8, 128, 64]
    idx = np.rint(idx).astype(np.int64)
    np.clip(idx, 0, K - 1, out=idx)
    # tile t on core c -> batch c*8 + t//8, tokens (t%8)*128 + p
    idx = idx.transpose(0, 2, 1).reshape(N_CORES, B_CORE, TILES_PER_BATCH,
                                         TOK_TILE)
    idx = idx.reshape(B_FULL, N)                              # [64, 1024]
    quant = cbk[idx]                                          # [64, 1024, 64]
    return np.ascontiguousarray(
        quant.transpose(0, 2, 1)).reshape(B_FULL, D, H, W)


if __name__ == "__main__":
    rng = np.random.default_rng(0)
    x = rng.standard_normal((B_FULL, D, H, W)).astype(np.float32)
    cbk = rng.standard_normal((K, D)).astype(np.float32)
    got = kernel(x, cbk)
    flat = x.reshape(B_FULL, D, N).transpose(0, 2, 1)
    dist = ((flat.astype(np.float64) ** 2).sum(-1, keepdims=True)
            - 2.0 * flat.astype(np.float64) @ cbk.T.astype(np.float64)
            + (cbk.astype(np.float64) ** 2).sum(-1))
    idx = dist.argmin(-1)
    exp = cbk[idx].transpose(0, 2, 1).reshape(B_FULL, D, H, W)
    err = np.linalg.norm(got - exp) / np.linalg.norm(exp)
    print("rel err vs numpy:", err)
    nflip = (got != exp).reshape(B_FULL, D, N).any(axis=1).sum()
    print("mismatched token count:", int(nflip), "/", B_FULL * N)


# revision 5
# speedup vs baseline: 2.0753x; 2.0753x over previous
"""VQ codebook kernel v4 for Trainium2, 8 NeuronCores.

Problem: x (64, 64, 32, 32) f32, codebook (512, 64) f32.
  idx = argmin_k ||flat - codebook[k]||^2 (first-min), out = codebook[idx].

Device computes argmin indices; host gathers codebook rows (0.01% of the
FLOPs, memory-bound on host, exact).

Sharding: data-parallel over batch (8 batches/core), codebook replicated.

Per-core pipeline, 64 token-tiles of 128 tokens, in quads (2x4 PSUM banks):
  PE:  score s = x.e - |e|^2/2 in bf16 hi/lo (3-term, proven <=1 flip):
       MM1 [xh;xl]x[eh;eh] + MM2a xh x el + MM2b ones2 x [e2a;e2b]
       (MM2a/b concurrent via tile_position row groups).
  DVE: r = prefix-max scan of s along k (tensor_tensor_scan, PSUM src,
       the ONLY DVE touch - no reduce, no match).
  ACT: idx = accum Sign(-r + r[:,511]) = #(k : r_k < max) = FIRST argmax,
       via the scalar engine's sum-accumulator. Exact (f32 compares,
       +-1/0 sums).
  One final DMA of idx [128, 64] f32 per core.
"""

import numpy as np
import ml_dtypes

import concourse.bass as bass
import concourse.mybir as mybir
import concourse.tile as tile
from concourse.bass_utils import run_bass_kernel_spmd

N_CORES = 8
B_FULL, D, H, W = 64, 64, 32, 32
N = H * W          # 1024 tokens per batch
K = 512            # codebook entries
B_CORE = B_FULL // N_CORES   # 8 batches per core
TOK_TILE = 128
TILES_PER_BATCH = N // TOK_TILE          # 8
N_TILES = B_CORE * TILES_PER_BATCH       # 64 per core
QUAD = 4

F32 = mybir.dt.float32
FP16 = mybir.dt.float16
BF16 = mybir.dt.bfloat16

MAX_SYNC_WAITS = 1


def _split_excess_waits(nc, max_waits=MAX_SYNC_WAITS):
    """This container's walrus rejects >2 sync waits per instruction; move
    excess waits onto InstNoOp instructions inserted just before."""
    for f in nc.m.functions:
        for bb in f.blocks:
            new_list = []
            for inst in bb.instructions:
                si = inst.sync_info
                if si is not None and si.on_wait and len(si.on_wait) > max_waits:
                    waits = list(si.on_wait)
                    extra, keep = waits[:-max_waits], waits[-max_waits:]
                    for i, w in enumerate(extra):
                        nop = mybir.InstNoOp(name=f"{inst.name}-sw{i}", ins=[], outs=[])
                        nop.engine = inst.engine
                        nop.sync_info = mybir.SyncInfo(on_wait=[w], on_update=[])
                        new_list.append(nop)
                    si.on_wait = keep
                    new_list.append(inst)
                else:
                    new_list.append(inst)
            bb.instructions[:] = new_list


def _emit_body(nc, pools, x_hl, r1, r2, ones2, ones32, cntbuf, rep):
    sb, dlp, ps = pools
    for q in range(N_TILES // QUAD):
        tiles = [QUAD * q + h for h in range(QUAD)]
        ps4 = ps.tile([TOK_TILE, QUAD, K], F32, tag="ps4")
        b, j0 = divmod(tiles[0], TILES_PER_BATCH)
        xt4 = sb.tile([128, QUAD * TOK_TILE], BF16, tag="xt4")
        nc.sync.dma_start(
            out=xt4[:], in_=x_hl[b, :, j0 * TOK_TILE:(j0 + QUAD) * TOK_TILE])
        for h, t in enumerate(tiles):
            xt = xt4[:, h * TOK_TILE:(h + 1) * TOK_TILE]
            nc.tensor.matmul(ps4[:, h, :], lhsT=xt, rhs=r1[:],
                             start=True, stop=False)
            nc.tensor.matmul(ps4[:, h, :], lhsT=xt[0:D, :], rhs=r2[0:D, :],
                             start=False, stop=False, tile_position=(0, 0))
            nc.tensor.matmul(ps4[:, h, :], lhsT=ones2[96:98, :],
                             rhs=r2[96:98, :],
                             start=False, stop=True, tile_position=(96, 0))

        for h, t in enumerate(tiles):
            r = dlp.tile([TOK_TILE, K], F32, tag="r")
            nc.vector.tensor_tensor_scan(
                out=r[:], data0=ps4[:, h, :], data1=ones32[:],
                initial=-3.0e38,
                op0=mybir.AluOpType.max, op1=mybir.AluOpType.mult)
            sg = dlp.tile([TOK_TILE, K], FP16, tag="sg")
            nc.scalar.activation(sg[:], r[:],
                                 mybir.ActivationFunctionType.Sign,
                                 bias=r[:, K - 1:K], scale=-1.0,
                                 accum_out=cntbuf[:, t:t + 1])


def build_nc(reps=1):
    nc = bass.Bass()
    x_hl = nc.dram_tensor("x_hl", [B_CORE, 128, N], BF16, kind="ExternalInput")
    r1d = nc.dram_tensor("r1d", [128, K], BF16, kind="ExternalInput")
    r2d = nc.dram_tensor("r2d", [98, K], BF16, kind="ExternalInput")
    out = nc.dram_tensor("out", [TOK_TILE, N_TILES], F32,
                         kind="ExternalOutput")

    with tile.TileContext(nc) as tc:
        with (
            tc.tile_pool(name="const", bufs=1) as constp,
            tc.tile_pool(name="sbuf", bufs=8) as sb,
            tc.tile_pool(name="delta", bufs=8) as dlp,
            tc.tile_pool(name="psum", bufs=2, space="PSUM") as ps,
        ):
            r1 = constp.tile([128, K], BF16)
            nc.sync.dma_start(out=r1[:], in_=r1d[:])
            r2 = constp.tile([98, K], BF16)
            nc.sync.dma_start(out=r2[:], in_=r2d[:])
            ones2 = constp.tile([98, TOK_TILE], BF16)
            nc.gpsimd.memset(ones2[:], 1.0)
            ones32 = constp.tile([TOK_TILE, K], F32)
            nc.gpsimd.memset(ones32[:], 1.0)

            pools = (sb, dlp, ps)
            for rep in range(reps):
                cntbuf = constp.tile([TOK_TILE, N_TILES], F32,
                                     tag=f"cntbuf{rep}")
                _emit_body(nc, pools, x_hl, r1, r2, ones2, ones32, cntbuf,
                           rep)
                nc.sync.dma_start(out=out[:], in_=cntbuf[:])

    _split_excess_waits(nc)
    return nc


_NC_CACHE = None


def _get_nc():
    global _NC_CACHE
    if _NC_CACHE is None:
        _NC_CACHE = build_nc()
    return _NC_CACHE


def prep_inputs(x, codebook):
    """Host-side prep: shard x over batch; bf16 hi/lo split of x; build
    rhs constants [eh;eh], [el ; pad ; e2a;e2b]."""
    bf = ml_dtypes.bfloat16
    x = np.asarray(x, dtype=np.float32).reshape(B_FULL, D, N)
    cbk = np.asarray(codebook, dtype=np.float32)

    eh = cbk.astype(bf).astype(np.float32)
    el = (cbk - eh).astype(bf).astype(np.float32)
    e2 = (cbk.astype(np.float64) ** 2).sum(-1)
    e2h32 = (-0.5 * e2).astype(np.float32)
    e2a = e2h32.astype(bf).astype(np.float32)
    e2b = (e2h32 - e2a).astype(np.float32)

    r1 = np.concatenate([eh.T, eh.T], axis=0).astype(bf)          # [128, K]
    r2 = np.zeros((98, K), dtype=np.float32)
    r2[0:D] = el.T
    r2[96] = e2a
    r2[97] = e2b
    r2 = r2.astype(bf)

    in_maps = []
    for c in range(N_CORES):
        xs = x[c * B_CORE:(c + 1) * B_CORE]                       # [8, 64, 1024]
        xh = xs.astype(bf).astype(np.float32)
        xl = (xs - xh)
        x_hl = np.concatenate([xh, xl], axis=1).astype(bf)        # [8, 128, 1024]
        in_maps.append({"x_hl": np.ascontiguousarray(x_hl), "r1d": r1,
                        "r2d": r2})
    return in_maps


def kernel(x, codebook):
    nc = _get_nc()
    cbk = np.asarray(codebook, dtype=np.float32)
    in_maps = prep_inputs(x, codebook)
    res = run_bass_kernel_spmd(nc, in_maps, core_ids=list(range(N_CORES)))
    # out[c] is [128, 64] f32: count (= argmin idx) of token p, tile t
    idx = np.stack([r["out"] for r in res.results], axis=0)   # [8, 128, 64]
    idx = np.rint(idx).astype(np.int64)
    np.clip(idx, 0, K - 1, out=idx)
    # tile t on core c -> batch c*8 + t//8, tokens (t%8)*128 + p
    idx = idx.transpose(0, 2, 1).reshape(N_CORES, B_CORE, TILES_PER_BATCH,
                                         TOK_TILE)
    idx = idx.reshape(B_FULL, N)                              # [64, 1024]
    quant = cbk[idx]                                          # [64, 1024, 64]
    return np.ascontiguousarray(
        quant.transpose(0, 2, 1)).reshape(B_FULL, D, H, W)


if __name__ == "__main__":
    rng = np.random.default_rng(0)
    x = rng.standard_normal((B_FULL, D, H, W)).astype(np.float32)
    cbk = rng.standard_normal((K, D)).astype(np.float32)
    got = kernel(x, cbk)
    flat = x.reshape(B_FULL, D, N).transpose(0, 2, 1)
    dist = ((flat.astype(np.float64) ** 2).sum(-1, keepdims=True)
            - 2.0 * flat.astype(np.float64) @ cbk.T.astype(np.float64)
            + (cbk.astype(np.float64) ** 2).sum(-1))
    idx = dist.argmin(-1)
    exp = cbk[idx].transpose(0, 2, 1).reshape(B_FULL, D, H, W)
    err = np.linalg.norm(got - exp) / np.linalg.norm(exp)
    print("rel err vs numpy:", err)
    nflip = (got != exp).reshape(B_FULL, D, N).any(axis=1).sum()
    print("mismatched token count:", int(nflip), "/", B_FULL * N)
